# revision 2
# baseline (speedup 1.0000x reference)
"""Trainium2 Bass kernel for BlockGivensRotation (w @ R, block-diagonal).

The reference applies, per 128-column block of w, 8 sequential sweeps of 127
adjacent-plane Givens rotations.  The composition of all 1016 rotations of a
block is a fixed 128x128 orthogonal matrix R_nb that depends only on `angles`,
so the whole op is `out[:, nb*128:(nb+1)*128] = w[:, nb*128:(nb+1)*128] @ R_nb`
- a block-diagonal matmul, ideal for the tensor engine.

Host side: compose R (tiny: 64x128x128, built in f64 from the 65K angles).
Device side: shard the 64 column-blocks across the 8 cores (8 blocks each) so
every core only needs its own slice of R.  Each core streams w.T tiles from
DRAM, matmuls with the per-block stationary R, and writes out.T tiles back.
w is fed transposed so the contraction dim (block columns) lies on SBUF
partitions with fully contiguous DMA; the host transposes shards in/out.

All HBM traffic and the matmul run in bf16 (PSUM accumulates f32): w rows are
iid randn and R is orthogonal, so quantizing w, R and out to bf16 costs
~2e-3 relative error against the 2e-2 gate while halving the 64.5 MB/core of
f32 I/O that bounds the fp32 version.  The kernel is DMA-bound (~32.25 MB of
HBM I/O per core), so the structure keeps the DMA queues saturated: w loads
on the SP HWDGE ring, R loads and out stores on the ACT ring, a halved first
tile so the PE starts early, and enough tile-pool depth that the PE never
starves.
"""

import numpy as np

import concourse.bacc as bacc
import concourse.mybir as mybir
import concourse.tile as tile
from concourse.bass_utils import run_bass_kernel_spmd

O = 8192          # w rows
IN_F = 8192       # w cols
B = 128           # Givens block size
NB = IN_F // B    # 64 blocks
NCORES = 8
BPC = NB // NCORES  # 8 column-blocks per core
F32 = mybir.dt.float32
BF16 = mybir.dt.bfloat16


def _build_rotation_matrices(angles: np.ndarray) -> np.ndarray:
    """Compose the sweeps of adjacent Givens rotations into one 128x128
    matrix per block by applying the reference recurrence to the identity
    (in float64, rounded once at the end)."""
    nb, s, bm1 = angles.shape
    b = bm1 + 1
    ang = np.asarray(angles, dtype=np.float64)
    c = np.cos(ang)
    sn = np.sin(ang)
    R = np.broadcast_to(np.eye(b), (nb, b, b)).copy()  # [NB, basis row, col]
    for sweep in range(s):
        cs, ss = c[:, sweep, :], sn[:, sweep, :]
        carry = R[:, :, 0].copy()
        for i in range(bm1):
            col_j = R[:, :, i + 1]
            ci = cs[:, i][:, None]
            si = ss[:, i][:, None]
            R[:, :, i] = ci * carry - si * col_j
            carry = si * carry + ci * col_j
        R[:, :, b - 1] = carry
    return R


def _build_bass(
    rows=O,
    bpc=BPC,
    ncores=NCORES,
    tile_rows=4096,
    wt_bufs=5,
    out_bufs=4,
    r_first=2,
    split_first=True,
):
    """Per-core program over this core's `bpc` column-blocks of w:

        out_t[blk*B + c', r] = sum_c R[blk][c, c'] * wt[blk*B + c, r]

    rows: w rows (full, 8192); tile_rows: rows per DMA tile;
    wt_bufs/out_bufs: pipeline depth; r_first: blocks of R in the first
    (small) R chunk so the first matmul isn't gated on the whole R slice;
    split_first: halve the first w tile so the PE starts sooner.
    """
    nc = bacc.Bacc(
        "TRN2", target_bir_lowering=False, debug=False, num_devices=ncores
    )
    wt = nc.dram_tensor("wt", [bpc * B, rows], BF16, kind="ExternalInput")
    r = nc.dram_tensor("r", [B, bpc * B], BF16, kind="ExternalInput")
    out_t = nc.dram_tensor("out_t", [bpc * B, rows], BF16, kind="ExternalOutput")

    hs = 512                    # moving free-dim per matmul (PSUM bank: 512 f32)

    with tile.TileContext(nc) as tc:
        with (
            tc.tile_pool(name="rp", bufs=1) as rp,
            tc.tile_pool(name="wtp", bufs=wt_bufs) as wtp,
            tc.tile_pool(name="outp", bufs=out_bufs) as outp,
            tc.tile_pool(name="psp", bufs=8, space="PSUM") as psp,
        ):
            # This core's R slice, in two chunks on the ACT ring so it
            # transfers in parallel with the first w tile on SP.
            rf = min(r_first, bpc)
            r_a = rp.tile([B, rf * B], BF16, tag="ra")
            nc.scalar.dma_start(r_a[:], r[:, : rf * B])
            r_b = None
            if rf < bpc:
                r_b = rp.tile([B, (bpc - rf) * B], BF16, tag="rb")
                nc.scalar.dma_start(r_b[:], r[:, rf * B :])
            for blk in range(bpc):
                if blk < rf:
                    r_ap = r_a[:, blk * B : (blk + 1) * B]
                else:
                    r_ap = r_b[:, (blk - rf) * B : (blk - rf + 1) * B]
                segs = [
                    (o, min(tile_rows, rows - o)) for o in range(0, rows, tile_rows)
                ]
                if split_first and blk == 0 and tile_rows >= 1024:
                    half = tile_rows // 2
                    segs = [(0, half), (half, half)] + segs[1:]
                for o, seg in segs:
                    wt_tile = wtp.tile([B, seg], BF16, tag="wt")
                    nc.sync.dma_start(
                        wt_tile[:], wt[blk * B : (blk + 1) * B, o : o + seg]
                    )
                    out_tile = outp.tile([B, seg], BF16, tag="out")
                    for h in range(seg // hs):
                        ps = psp.tile([B, hs], F32)
                        nc.tensor.matmul(
                            ps[:],
                            r_ap,
                            wt_tile[:, h * hs : (h + 1) * hs],
                            start=True,
                            stop=True,
                        )
                        nc.vector.tensor_copy(
                            out_tile[:, h * hs : (h + 1) * hs], ps[:]
                        )
                    # out-stores ride the second HWDGE ring (ACT)
                    nc.scalar.dma_start(
                        out_t[blk * B : (blk + 1) * B, o : o + seg], out_tile[:]
                    )
    nc.compile()
    return nc


def kernel_impl(w, angles, trace=False, bass_kwargs=None, **spmd_kwargs):
    import ml_dtypes

    bf16 = ml_dtypes.bfloat16
    w = np.asarray(w)
    Rm = _build_rotation_matrices(np.asarray(angles))
    # r_host[c, blk*B + c'] = R[blk][c, c']  (contiguous per SBUF partition c)
    r_host = (
        np.ascontiguousarray(Rm.transpose(1, 0, 2)).reshape(B, NB * B).astype(bf16)
    )
    w_bf = w.astype(bf16)
    nc = _build_bass(**(bass_kwargs or {}))
    csz = BPC * B  # 1024 w-columns per core
    in_maps = [
        {
            "wt": w_bf[:, i * csz : (i + 1) * csz].T,
            "r": r_host[:, i * csz : (i + 1) * csz],
        }
        for i in range(NCORES)
    ]
    res = run_bass_kernel_spmd(
        nc, in_maps, core_ids=list(range(NCORES)), trace=trace, **spmd_kwargs
    )
    out = np.empty((O, IN_F), dtype=np.float32)
    for i in range(NCORES):
        out[:, i * csz : (i + 1) * csz] = res.results[i]["out_t"].T.astype(np.float32)
    return out, res


def kernel(w, angles):
    out, _ = kernel_impl(w, angles, trace=False)
    return out


# revision 5
# speedup vs baseline: 1.9590x; 1.9590x over previous
"""Trainium2 Bass kernel for BlockGivensRotation (w @ R, block-diagonal).

The reference applies, per 128-column block of w, 8 sequential sweeps of 127
adjacent-plane Givens rotations.  The composition of all 1016 rotations of a
block is a fixed 128x128 orthogonal matrix R_nb that depends only on `angles`,
so the whole op is `out[:, nb*128:(nb+1)*128] = w[:, nb*128:(nb+1)*128] @ R_nb`
- a block-diagonal matmul, ideal for the tensor engine.

Host side: compose R (tiny: 64x128x128, built in f64 from the 65K angles).
Device side: shard the 64 column-blocks across the 8 cores (8 blocks each) so
every core only needs its own slice of R.  Each core streams w.T tiles from
DRAM, matmuls with the per-block stationary R, and writes out.T tiles back.
w is fed transposed so the contraction dim (block columns) lies on SBUF
partitions with fully contiguous DMA; the host transposes shards in/out.

All HBM traffic and the matmul run in bf16 (PSUM accumulates f32): w rows are
iid randn and R is orthogonal, so quantizing w, R and out to bf16 costs
~2e-3 relative error against the 2e-2 gate while halving the 64.5 MB/core of
f32 I/O that bounds the fp32 version.  The kernel is DMA-bound (~32.25 MB of
HBM I/O per core), so the structure keeps the DMA queues saturated: w loads
on the SP HWDGE ring, R loads and out stores on the ACT ring, a halved first
tile so the PE starts early, and enough tile-pool depth that the PE never
starves.
"""

import numpy as np

import concourse.bacc as bacc
import concourse.mybir as mybir
import concourse.tile as tile
from concourse.bass_utils import run_bass_kernel_spmd

O = 8192          # w rows
IN_F = 8192       # w cols
B = 128           # Givens block size
NB = IN_F // B    # 64 blocks
NCORES = 8
BPC = NB // NCORES  # 8 column-blocks per core
F32 = mybir.dt.float32
BF16 = mybir.dt.bfloat16


def _build_rotation_matrices(angles: np.ndarray) -> np.ndarray:
    """Compose the sweeps of adjacent Givens rotations into one 128x128
    matrix per block by applying the reference recurrence to the identity
    (in float64, rounded once at the end)."""
    nb, s, bm1 = angles.shape
    b = bm1 + 1
    ang = np.asarray(angles, dtype=np.float64)
    c = np.cos(ang)
    sn = np.sin(ang)
    R = np.broadcast_to(np.eye(b), (nb, b, b)).copy()  # [NB, basis row, col]
    for sweep in range(s):
        cs, ss = c[:, sweep, :], sn[:, sweep, :]
        carry = R[:, :, 0].copy()
        for i in range(bm1):
            col_j = R[:, :, i + 1]
            ci = cs[:, i][:, None]
            si = ss[:, i][:, None]
            R[:, :, i] = ci * carry - si * col_j
            carry = si * carry + ci * col_j
        R[:, :, b - 1] = carry
    return R


def _build_bass(
    rows=O,
    bpc=BPC,
    ncores=NCORES,
    tile_rows=4096,
    wt_bufs=5,
    out_bufs=4,
    r_first=2,
    split_first=True,
):
    """Per-core program over this core's `bpc` column-blocks of w:

        out_t[blk*B + c', r] = sum_c R[blk][c, c'] * wt[blk*B + c, r]

    rows: w rows (full, 8192); tile_rows: rows per DMA tile;
    wt_bufs/out_bufs: pipeline depth; r_first: blocks of R in the first
    (small) R chunk so the first matmul isn't gated on the whole R slice;
    split_first: halve the first w tile so the PE starts sooner.
    """
    nc = bacc.Bacc(
        "TRN2", target_bir_lowering=False, debug=False, num_devices=ncores
    )
    wt = nc.dram_tensor("wt", [bpc * B, rows], BF16, kind="ExternalInput")
    r = nc.dram_tensor("r", [B, bpc * B], BF16, kind="ExternalInput")
    out_t = nc.dram_tensor("out_t", [bpc * B, rows], BF16, kind="ExternalOutput")

    hs = 512                    # moving free-dim per matmul (PSUM bank: 512 f32)

    with tile.TileContext(nc) as tc:
        with (
            tc.tile_pool(name="rp", bufs=1) as rp,
            tc.tile_pool(name="wtp", bufs=wt_bufs) as wtp,
            tc.tile_pool(name="outp", bufs=out_bufs) as outp,
            tc.tile_pool(name="psp", bufs=8, space="PSUM") as psp,
        ):
            # This core's R slice, in two chunks on the ACT ring so it
            # transfers in parallel with the first w tile on SP.
            rf = min(r_first, bpc)
            r_a = rp.tile([B, rf * B], BF16, tag="ra")
            nc.scalar.dma_start(r_a[:], r[:, : rf * B])
            r_b = None
            if rf < bpc:
                r_b = rp.tile([B, (bpc - rf) * B], BF16, tag="rb")
                nc.scalar.dma_start(r_b[:], r[:, rf * B :])
            for blk in range(bpc):
                if blk < rf:
                    r_ap = r_a[:, blk * B : (blk + 1) * B]
                else:
                    r_ap = r_b[:, (blk - rf) * B : (blk - rf + 1) * B]
                segs = [
                    (o, min(tile_rows, rows - o)) for o in range(0, rows, tile_rows)
                ]
                if split_first and blk == 0 and tile_rows >= 1024:
                    half = tile_rows // 2
                    segs = [(0, half), (half, half)] + segs[1:]
                for o, seg in segs:
                    wt_tile = wtp.tile([B, seg], BF16, tag="wt")
                    nc.sync.dma_start(
                        wt_tile[:], wt[blk * B : (blk + 1) * B, o : o + seg]
                    )
                    out_tile = outp.tile([B, seg], BF16, tag="out")
                    for h in range(seg // hs):
                        ps = psp.tile([B, hs], F32)
                        nc.tensor.matmul(
                            ps[:],
                            r_ap,
                            wt_tile[:, h * hs : (h + 1) * hs],
                            start=True,
                            stop=True,
                        )
                        # The f32->bf16 PSUM drain is ~88us on DVE alone;
                        # split it with ACT (GPSIMD cannot access PSUM).
                        dst = out_tile[:, h * hs : (h + 1) * hs]
                        if h % 2 == 0:
                            nc.vector.tensor_copy(dst, ps[:])
                        else:
                            nc.scalar.copy(dst, ps[:])
                    # out-stores ride the second HWDGE ring (ACT)
                    nc.scalar.dma_start(
                        out_t[blk * B : (blk + 1) * B, o : o + seg], out_tile[:]
                    )
    nc.compile()
    return nc


def kernel_impl(w, angles, trace=False, bass_kwargs=None, **spmd_kwargs):
    import ml_dtypes

    bf16 = ml_dtypes.bfloat16
    w = np.asarray(w)
    Rm = _build_rotation_matrices(np.asarray(angles))
    # r_host[c, blk*B + c'] = R[blk][c, c']  (contiguous per SBUF partition c)
    r_host = (
        np.ascontiguousarray(Rm.transpose(1, 0, 2)).reshape(B, NB * B).astype(bf16)
    )
    w_bf = w.astype(bf16)
    nc = _build_bass(**(bass_kwargs or {}))
    csz = BPC * B  # 1024 w-columns per core
    in_maps = [
        {
            "wt": w_bf[:, i * csz : (i + 1) * csz].T,
            "r": r_host[:, i * csz : (i + 1) * csz],
        }
        for i in range(NCORES)
    ]
    res = run_bass_kernel_spmd(
        nc, in_maps, core_ids=list(range(NCORES)), trace=trace, **spmd_kwargs
    )
    out = np.empty((O, IN_F), dtype=np.float32)
    for i in range(NCORES):
        out[:, i * csz : (i + 1) * csz] = res.results[i]["out_t"].T.astype(np.float32)
    return out, res


def kernel(w, angles):
    out, _ = kernel_impl(w, angles, trace=False)
    return out
